# revision 44
# baseline (speedup 1.0000x reference)
"""BartLatentAttention Trainium2 kernel.

Full-input contract: kernel(**inputs) takes the unsharded tensors from
setup_inputs() and returns the full [B, T, D] float32 output.

Sharding: tensor-parallel over heads. 16 heads / 8 cores = 2 heads per
core. Each core computes q/k/v projections for its 2 heads (column-sliced
weights), attention over the latent-prefixed sequence, and a partial
output projection (row-sliced Wo). The host sums the 8 partial outputs
and adds bo.

Device-side layout notes:
  - hidden is fed pre-transposed as hT [D, B*T] bf16 so projections can
    stream it as the moving matmul operand (contraction over D on the
    partition axis).
  - scores are computed transposed (scoresT [s, t]) so that softmax's
    exp can run on ScalarE straight out of PSUM, and the AV matmul can
    consume expT as the moving operand with V [s, d] stationary. The two
    heads' score matmuls land on PE row-tiles (0,0)/(64,0) and execute
    concurrently.
  - V carries an extra ones-column (M=65): PSUM row 64 of the AV
    accumulation is the softmax denominator Z for free.
  - S = L + T = 2056 is laid out padded to 2176 = 17*128: chunk 0 holds
    the 8 latent positions + 120 dead rows (killed with an exp bias of
    -30), chunks 1..16 hold the 2048 token positions.
  - epilogue is all-bf16 (zb broadcast matmul, ot, wo) and the final
    out-projection tiles are DMA'd directly from PSUM to DRAM.
  - ~28 junk 128-col matmuls at t=0 trip the PE HAM activity monitor so
    the real work starts at 2.4 GHz instead of 1.2 GHz.
"""

import sys

if "/opt/trn_rl_repo" not in sys.path:
    sys.path.insert(0, "/opt/trn_rl_repo")

import numpy as np
import ml_dtypes

BF16 = ml_dtypes.bfloat16

B, T, D = 2, 2048, 1024
H = 16
HD = D // H  # 64
L = 8
S = L + T  # 2056
SCALE = HD ** -0.5
NCORES = 8
HPC = H // NCORES  # heads per core = 2
DC = HPC * HD  # per-core feature width = 128

BT = B * T  # 4096
NKC = D // 128  # k chunks = 8
NTC = BT // 512  # token chunks of 512 = 8
SCHUNKS = 17  # padded S = 2176 = 17 * 128
TB = 512  # attention t-block
NTB = T // TB  # 4 per batch
PAD_BIAS = -30.0

_cache: dict = {}


def _build_nc():
    import concourse.bass as bass
    import concourse.mybir as mybir
    import concourse.tile as tile
    from concourse import bacc

    fp32 = mybir.dt.float32
    bf16 = mybir.dt.bfloat16

    nc = bacc.Bacc(
        "TRN2",
        target_bir_lowering=False,
        debug=False,
        enable_asserts=False,
        num_devices=NCORES,
    )

    # DRAM I/O. hT arrives pre-tiled [g, p, k, t] so each half-chunk DMA
    # reads one fully contiguous 256 KiB block.
    hT = nc.dram_tensor("hT", [NTC, 128, NKC, 512], bf16,
                        kind="ExternalInput").ap()
    wq = nc.dram_tensor("wq", [D, DC], bf16, kind="ExternalInput").ap()
    wk = nc.dram_tensor("wk", [D, DC], bf16, kind="ExternalInput").ap()
    wv = nc.dram_tensor("wv", [D, DC], bf16, kind="ExternalInput").ap()
    bq = nc.dram_tensor("bq", [DC, 1], fp32, kind="ExternalInput").ap()
    bk = nc.dram_tensor("bk", [DC, 1], fp32, kind="ExternalInput").ap()
    bv1 = nc.dram_tensor("bv1", [DC, 1], fp32, kind="ExternalInput").ap()
    wo = nc.dram_tensor("wo", [DC, D], bf16, kind="ExternalInput").ap()
    lkT = nc.dram_tensor("lkT", [B, DC, L], bf16, kind="ExternalInput").ap()
    lv = nc.dram_tensor("lv", [B, HPC, L, HD], bf16, kind="ExternalInput").ap()
    ebias0 = nc.dram_tensor("ebias0", [128, 1], fp32, kind="ExternalInput").ap()
    e2 = nc.dram_tensor("e2", [2, 128], bf16, kind="ExternalInput").ap()
    fp16 = mybir.dt.float16
    out = nc.dram_tensor("out", [BT, D], fp16, kind="ExternalOutput").ap()

    EXP = mybir.ActivationFunctionType.Exp

    with tile.TileContext(nc) as tc:
        with (
            tc.tile_pool(name="consts", bufs=1) as consts,
            tc.tile_pool(name="persist", bufs=1) as persist,
            tc.tile_pool(name="htiles", bufs=2) as htiles,
            tc.tile_pool(name="exps", bufs=6) as exps,
            tc.tile_pool(name="episb", bufs=2) as episb,
            tc.tile_pool(name="scps", bufs=2, space="PSUM") as scps,
            tc.tile_pool(name="avp", bufs=2, space="PSUM") as avp,
            tc.tile_pool(name="mps", bufs=2, space="PSUM") as mps,
        ):
            # ---- PE warm-up: identity + junk matmuls, no DMA deps ----
            ident = consts.tile([128, 128], bf16)
            from concourse.masks import make_identity
            make_identity(nc, ident)
            # ~40 junk matmuls ≈ 4.3us of cold-clock busy: trips the HAM
            # activity monitor so the real phase-1 work runs at 2.4 GHz
            dum = mps.tile([128, 512], fp32, tag="mm", name="dum")
            for i in range(40):
                nc.tensor.matmul(dum[:, 0:128], ident, ident,
                                 start=True, stop=True)

            # ---- constants (DMA order matters: qkv weights + first h
            # chunk first, wo last) ----
            wq_sb = consts.tile([128, NKC * DC], bf16)  # [128, 8*128]
            wk_sb = consts.tile([128, NKC * DC], bf16)
            wv_sb = consts.tile([128, NKC * DC], bf16)
            wq_v = wq_sb.rearrange("p (k c) -> p k c", k=NKC)
            wk_v = wk_sb.rearrange("p (k c) -> p k c", k=NKC)
            wv_v = wv_sb.rearrange("p (k c) -> p k c", k=NKC)
            bq_sb = consts.tile([DC, 1], fp32)
            bk_sb = consts.tile([DC, 1], fp32)
            bv1_sb = consts.tile([DC, 1], fp32)
            wo_sb = consts.tile([DC, D], bf16)
            eb0_sb = consts.tile([128, 1], fp32)
            e2_sb = consts.tile([2, 128], bf16)
            # weights ride the otherwise-idle scalar DMA ring; small
            # constants go on gpsimd; the big hT loads are split in half
            # across the sync+gpsimd rings
            nc.scalar.dma_start(out=wq_v, in_=wq.rearrange("(k p) c -> p k c", p=128))
            nc.scalar.dma_start(out=eb0_sb, in_=ebias0)
            nc.scalar.dma_start(out=wk_v, in_=wk.rearrange("(k p) c -> p k c", p=128))
            nc.scalar.dma_start(out=wv_v, in_=wv.rearrange("(k p) c -> p k c", p=128))
            nc.gpsimd.dma_start(out=bq_sb, in_=bq)
            nc.gpsimd.dma_start(out=bk_sb, in_=bk)
            nc.gpsimd.dma_start(out=bv1_sb, in_=bv1)
            nc.gpsimd.dma_start(out=e2_sb, in_=e2)

            # ---- persistent activations ----
            qT_sb = persist.tile([128, BT], bf16)  # [h0|h1 feats, global tok]
            kT_sb = persist.tile([128, B * SCHUNKS * 128], bf16)  # per b: 2176
            v_sb = persist.tile([128, B * HPC * SCHUNKS * 65], bf16)

            def k_off(b):
                return b * SCHUNKS * 128

            def v_off(b, h, c):
                return ((b * HPC + h) * SCHUNKS + c) * 65

            # latent / pad setup
            for b in range(B):
                nc.vector.memset(kT_sb[:, k_off(b) + L:k_off(b) + 128], 0.0)
                nc.gpsimd.dma_start(out=kT_sb[:, k_off(b):k_off(b) + L],
                                    in_=lkT[b])
                for h in range(HPC):
                    nc.vector.memset(
                        v_sb[:, v_off(b, h, 0):v_off(b, h, 0) + 65], 0.0)
            # ones column for the Z fold (col 64 of every [128, 65] chunk)
            v_view = v_sb.rearrange("p (n c) -> p n c", c=65)
            nc.vector.memset(v_view[:, :, 64:65], 1.0)
            for b in range(B):
                for h in range(HPC):
                    nc.gpsimd.dma_start(
                        out=v_sb[0:L, v_off(b, h, 0):v_off(b, h, 0) + HD],
                        in_=lv[b, h])

            # wo loads on the scalar ring behind the qkv weights (not
            # needed until the first epilogue)
            def load_wo():
                nc.scalar.dma_start(out=wo_sb, in_=wo)

            # ---- qkv projection closures for one 512-token chunk ----
            def qkv_closures(g):
                t0g = g * 512
                bb = t0g // T
                c0 = (t0g - bb * T) // 128 + 1
                hold = {}

                def ht_load():
                    ht = htiles.tile([128, NKC, 512], bf16, tag="ht",
                                     name=f"ht_{g}")
                    # k-chunks 0-3 are consumed first (qa/ka/va); load the
                    # two halves on different DMA rings so each g-chunk
                    # lands in half the time. The scalar ring helps out in
                    # phase 1 while it has no activations to run.
                    e0, e1 = {0: (nc.sync, nc.gpsimd),
                              1: (nc.scalar, nc.sync),
                              2: (nc.gpsimd, nc.scalar),
                              3: (nc.sync, nc.gpsimd),
                              4: (nc.gpsimd, nc.sync),
                              5: (nc.sync, nc.gpsimd),
                              6: (nc.gpsimd, nc.sync),
                              7: (nc.sync, nc.gpsimd)}[g]
                    e0.dma_start(out=ht[:, 0:4, :], in_=hT[g, :, 0:4, :])
                    e1.dma_start(out=ht[:, 4:8, :], in_=hT[g, :, 4:8, :])
                    hold["ht"] = ht

                def mk_proj(key, w_v):
                    def pa():
                        ps = mps.tile([128, 512], fp32, tag="mm",
                                      name=f"{key}ps_{g}")
                        hold[key] = ps
                        for k in range(4):
                            nc.tensor.matmul(
                                ps, w_v[:, k, :], hold["ht"][:, k, :],
                                start=(k == 0), stop=False)

                    def pb():
                        ps = hold[key]
                        for k in range(4, NKC):
                            nc.tensor.matmul(
                                ps, w_v[:, k, :], hold["ht"][:, k, :],
                                start=False, stop=(k == NKC - 1))
                    return pa, pb

                qa, qb = mk_proj("q", wq_v)
                ka, kb = mk_proj("k", wk_v)
                va, vb = mk_proj("v", wv_v)

                def q_fin():
                    nc.vector.tensor_scalar_add(
                        qT_sb[:, t0g:t0g + 512], hold["q"], bq_sb)

                def k_fin():
                    koff = k_off(bb) + 128 + (t0g - bb * T)
                    nc.vector.tensor_scalar_add(
                        kT_sb[:, koff:koff + 512], hold["k"], bk_sb)

                def v_fin():
                    vt = episb.tile([128, 512], bf16, tag="vt",
                                    name=f"vt_{g}")
                    nc.vector.tensor_scalar_add(vt, hold["v"], bv1_sb)
                    hold["vt"] = vt

                def t_a():
                    tp = mps.tile([128, 512], bf16, tag="mm",
                                  name=f"tp_{g}")
                    hold["tp"] = tp
                    for j in range(2):
                        nc.tensor.transpose(
                            tp[:, j * 128:(j + 1) * 128],
                            hold["vt"][:, j * 128:(j + 1) * 128], ident)

                def t_b():
                    tp = hold["tp"]
                    for j in range(2, 4):
                        nc.tensor.transpose(
                            tp[:, j * 128:(j + 1) * 128],
                            hold["vt"][:, j * 128:(j + 1) * 128], ident)
                    # v_sb[:, (c0+m, h, d)] = tp[:, (m, h, d)]
                    dst = bass.AP(
                        tensor=v_sb.tensor,
                        offset=v_sb.offset + v_off(bb, 0, c0),
                        ap=[v_sb.ap[0], [65, 4], [SCHUNKS * 65, HPC],
                            [1, HD]])
                    srcv = tp.rearrange("p (m e) -> p m e", m=4)
                    src = bass.AP(
                        tensor=srcv.tensor, offset=srcv.offset,
                        ap=[srcv.ap[0], [128, 4], [64, HPC], [1, HD]])
                    nc.vector.tensor_copy(dst, src)

                return [ht_load, qa, qb, q_fin, ka, kb, k_fin,
                        va, vb, v_fin, t_a, t_b]

            # ---- attention helpers ----
            def emit_epi_drain(st, last=False):
                av0, av1, tw = st["av0"], st["av1"], st["tw"]
                oz = episb.tile([128, 512], bf16, tag="oz",
                                name=f"oz_{st['q0']}")
                zr2 = episb.tile([2, 512], bf16, tag="zr2",
                                 name=f"zr2_{st['q0']}")
                zh0 = episb.tile([1, 512], fp32, tag="zh0",
                                 name=f"zh0_{st['q0']}")
                zh1 = episb.tile([1, 512], fp32, tag="zh1",
                                 name=f"zh1_{st['q0']}")
                rz1 = episb.tile([1, 512], bf16, tag="rz1",
                                 name=f"rz1_{st['q0']}")
                # the zb matmul waits on zr2, so emit that chain first;
                # the oz evacuation follows
                nc.vector.tensor_copy(zh0[:, :tw], av0[64:65, :tw])
                nc.vector.tensor_copy(zh1[:, :tw], av1[64:65, :tw])
                nc.vector.reciprocal_approx_fast(out=zh0[:, :tw],
                                                 in_=zh0[:, :tw])
                nc.vector.reciprocal_approx_fast(out=zh1[:, :tw],
                                                 in_=zh1[:, :tw])
                # DVE writes must start at a 32-aligned partition; row 1 of
                # zr2 goes through a DMA instead
                nc.vector.tensor_copy(zr2[0:1, :tw], zh0[:, :tw])
                nc.vector.tensor_copy(rz1[:, :tw], zh1[:, :tw])
                nc.gpsimd.dma_start(out=zr2[1:2, :tw], in_=rz1[:, :tw])
                nc.vector.tensor_copy(oz[0:64, :tw], av0[0:64, :tw])
                nc.vector.tensor_copy(oz[64:128, :tw], av1[0:64, :tw])
                st["oz"], st["zr2"] = oz, zr2
                if last:
                    # chained junk matmuls bridge the drain latency so the
                    # PE activity monitor keeps the clock at 2.4 GHz into
                    # the final epilogue
                    for r in range(4):
                        jk = avp.tile([65, 512], fp32, tag="avp",
                                      name=f"jkd_{st['q0']}_{r}")
                        nc.tensor.matmul(jk[:, :tw], v_sb[:, 0:65],
                                         oz[:, :tw], start=True, stop=True)

            def emit_epi_zb(st):
                tw = st["tw"]
                zb = mps.tile([128, 512], fp32, tag="mm",
                              name=f"zb_{st['q0']}")
                nc.tensor.matmul(zb[:, :tw], e2_sb, st["zr2"][:, :tw],
                                 start=True, stop=True)
                ot = episb.tile([128, 512], bf16, tag="ot",
                                name=f"ot_{st['q0']}")
                nc.vector.tensor_mul(ot[:, :tw], st["oz"][:, :tw],
                                     zb[:, :tw])
                st["ot"] = ot

            def mk_epi_out(st, j, last=False):
                def go():
                    if j >= st["tw"] // 128:
                        return
                    ot, q0 = st["ot"], st["q0"]
                    r0 = q0 + j * 128
                    for f in range(2):
                        # during the final drain the score PSUM banks are
                        # free: alternate pools for twice the pipeline
                        # depth, so op matmuls don't serialize on casts
                        if last and f == 1:
                            op = scps.tile([128, 1024], fp32, tag="sc",
                                           name=f"op_{q0}_{j}_{f}")
                            op = op[:, 0:512]
                        else:
                            op = mps.tile([128, 512], fp32, tag="mm",
                                          name=f"op_{q0}_{j}_{f}")
                        nc.tensor.matmul(
                            op, ot[:, j * 128:(j + 1) * 128],
                            wo_sb[:, f * 512:(f + 1) * 512],
                            start=True, stop=True)
                        osb = episb.tile([128, 512], fp16, tag="osb",
                                         name=f"osb_{q0}_{j}_{f}")
                        nc.vector.tensor_copy(osb, op)
                        nc.sync.dma_start(
                            out=out[r0:r0 + 128, f * 512:(f + 1) * 512],
                            in_=osb)
                        if last:
                            # junk matmul chained on the cast keeps the
                            # PE activity monitor from downclocking
                            jk = avp.tile([65, 512], fp32, tag="avp",
                                          name=f"jk_{q0}_{j}_{f}")
                            nc.tensor.matmul(
                                jk, v_sb[:, 0:65], osb,
                                start=True, stop=True)
                return go

            def emit_av(st, c):
                b, tw = st["b"], st["tw"]
                stt, sp = c == 0, c == SCHUNKS - 1
                ex = st["ex"].pop(c)
                for h, av in ((0, st["av0"]), (1, st["av1"])):
                    vo = v_off(b, h, c)
                    eh = ex[:, h * tw:(h + 1) * tw]
                    nc.tensor.matmul(
                        av[:, :tw], v_sb[:, vo:vo + 65], eh,
                        start=stt, stop=sp)

            def make_st(b, q0, tw=TB):
                return {
                    "b": b, "q0": q0, "tw": tw,
                    "av0": avp.tile([65, 512], fp32, tag="avp",
                                    name=f"av0_{q0}"),
                    "av1": avp.tile([65, 512], fp32, tag="avp",
                                    name=f"av1_{q0}"),
                    "ex": {},
                }

            def chunk_body(st, c):
                b, q0, tw = st["b"], st["q0"], st["tw"]
                sc = scps.tile([128, 1024], fp32, tag="sc",
                               name=f"sc_{b}_{q0}_{c}")
                kc = k_off(b) + c * 128
                nc.tensor.matmul(
                    sc[:, 0:tw],
                    kT_sb[0:64, kc:kc + 128],
                    qT_sb[0:64, q0:q0 + tw],
                    start=True, stop=True)
                nc.tensor.matmul(
                    sc[:, tw:2 * tw],
                    kT_sb[64:128, kc:kc + 128],
                    qT_sb[64:128, q0:q0 + tw],
                    start=True, stop=True)
                ex = exps.tile([128, 1024], bf16, tag="ex",
                               name=f"ex_{b}_{q0}_{c}")
                nc.scalar.activation(
                    ex[:, 0:2 * tw], sc[:, 0:2 * tw], EXP,
                    bias=(eb0_sb if c == 0 else 0.0), scale=1.0)
                st["ex"][c] = ex
                if c >= 1:
                    emit_av(st, c - 1)

            def queue_epilogue(st, side, last=False):
                emit_av(st, SCHUNKS - 1)
                emit_epi_drain(st, last)
                # queued to run inside the next tb; two no-op slots let
                # the drain chain finish before zb consumes zr2

                def mk_zb(s):
                    def go():
                        emit_epi_zb(s)
                    return go
                noop = lambda: None
                for cl in reversed([noop, noop, mk_zb(st),
                                    mk_epi_out(st, 0, last),
                                    mk_epi_out(st, 1, last),
                                    mk_epi_out(st, 2, last),
                                    mk_epi_out(st, 3, last)]):
                    side.appendleft(cl)

            # ---- phase 1: qkv for batch 0, interleaved with the first
            # t-block's attention chunks (each chunk's K/V comes from the
            # previous g-chunk, so tb0 trails the projections by one) ----
            from collections import deque
            side = deque()
            for cl in qkv_closures(0):
                cl()
            load_wo()
            st0 = make_st(0, 0)
            chunk_body(st0, 0)
            for g in (1, 2, 3):
                cls = qkv_closures(g)
                base = 4 * (g - 1)
                for i in range(4):
                    chunk_body(st0, base + 1 + i)
                    for cl in cls[3 * i:3 * i + 3]:
                        cl()
            for c in range(13, SCHUNKS):
                chunk_body(st0, c)
            queue_epilogue(st0, side)
            for g in range(NTC // 2, NTC):
                side.extend(qkv_closures(g))

            # ---- phase 2: remaining t-blocks with interleaved side work
            # (tb0's epilogue, qkv for batch 1, later epilogues). The very
            # last t-block is split in half so most of its epilogue
            # overlaps the second half's chunk pipeline. ----
            schedule = ([(0, q, TB) for q in range(TB, T, TB)] +
                        [(1, T + q, TB) for q in range(0, T, TB)])
            for bi, (b, q0, tw) in enumerate(schedule):
                st = make_st(b, q0, tw)
                for c in range(SCHUNKS):
                    chunk_body(st, c)
                    if c >= 1 and side:
                        side.popleft()()
                        # drain two at a time while the backlog is
                        # long so batch 1's K/V is ready in time
                        if len(side) > 26 and side:
                            side.popleft()()
                queue_epilogue(st, side, last=(bi == len(schedule) - 1))
            # flush remaining side work (the last epilogues)
            while side:
                side.popleft()()

    nc.compile()
    return nc


def _get_nc():
    if "nc" not in _cache:
        _cache["nc"] = _build_nc()
    return _cache["nc"]


def _prep_inputs(hidden_states, decoder_latent, Wq, bq, Wk, bk, Wv, bv, Wo):
    """Build the 8 per-core input maps (host-side sharding/layout)."""
    # pre-tiled hT: element (g, p, k, t) = hidden[g*512+t, k*128+p]
    hh = hidden_states.reshape(BT, D).T.astype(BF16)  # [D, BT]
    hT = np.ascontiguousarray(
        hh.reshape(NKC, 128, NTC, 512).transpose(2, 1, 0, 3))
    lk = decoder_latent[..., :HD]  # [B, H, L, HD]
    lvf = decoder_latent[..., HD:]
    eb0 = np.full((128, 1), PAD_BIAS, np.float32)
    eb0[:L] = 0.0
    e2 = np.zeros((2, 128), np.float32)
    e2[0, 0:64] = 1.0
    e2[1, 64:128] = 1.0
    in_maps = []
    for c in range(NCORES):
        cols = slice(c * DC, (c + 1) * DC)
        h0, h1 = HPC * c, HPC * c + 1
        lkT_c = np.stack([
            np.concatenate([lk[b, h0].T, lk[b, h1].T], axis=0)
            for b in range(B)])  # [B, 128, L]
        in_maps.append({
            "hT": hT,
            "wq": (Wq[:, cols] * SCALE).astype(BF16),
            "wk": Wk[:, cols].astype(BF16),
            "wv": Wv[:, cols].astype(BF16),
            "bq": (bq[cols] * SCALE).astype(np.float32).reshape(DC, 1),
            "bk": bk[cols].astype(np.float32).reshape(DC, 1),
            "bv1": bv[cols].astype(np.float32).reshape(DC, 1),
            "wo": Wo[cols, :].astype(BF16),
            "lkT": lkT_c.astype(BF16),
            "lv": lvf[:, h0:h1 + 1].astype(BF16),
            "ebias0": eb0,
            "e2": e2.astype(BF16),
        })
    return in_maps


def _run(inputs, trace=False):
    from concourse.bass_utils import run_bass_kernel_spmd

    nc = _get_nc()
    in_maps = _prep_inputs(
        inputs["hidden_states"], inputs["decoder_latent"],
        inputs["Wq"], inputs["bq"], inputs["Wk"], inputs["bk"],
        inputs["Wv"], inputs["bv"], inputs["Wo"])
    res = run_bass_kernel_spmd(nc, in_maps, core_ids=list(range(NCORES)),
                               trace=trace)
    acc = np.zeros((BT, D), np.float64)
    for r in res.results:
        acc += r["out"].astype(np.float64)
    out = (acc + inputs["bo"].astype(np.float64)).astype(np.float32)
    return out.reshape(B, T, D), res


def _reference_fallback(hidden_states, decoder_latent, attention_mask,
                        Wq, bq, Wk, bk, Wv, bv, Wo, bo):
    """Exact numpy path, used only when attention_mask is non-zero (the
    problem spec fills it with zeros; the device kernel specializes on
    that)."""
    x = hidden_states.astype(np.float64)
    q = (x @ Wq + bq) * SCALE
    k = x @ Wk + bk
    v = x @ Wv + bv

    def heads(a):
        return a.reshape(B, T, H, HD).transpose(0, 2, 1, 3)

    q, k, v = heads(q), heads(k), heads(v)
    lk = decoder_latent[..., :HD].astype(np.float64)
    lv = decoder_latent[..., HD:].astype(np.float64)
    k = np.concatenate([lk, k], axis=2)
    v = np.concatenate([lv, v], axis=2)
    s = np.einsum("bhtd,bhsd->bhts", q, k) + attention_mask.astype(np.float64)
    s -= s.max(axis=-1, keepdims=True)
    p = np.exp(s)
    p /= p.sum(axis=-1, keepdims=True)
    o = np.einsum("bhts,bhsd->bhtd", p, v)
    o = o.transpose(0, 2, 1, 3).reshape(B, T, D)
    return (o @ Wo + bo).astype(np.float32)


def kernel(**inputs):
    inputs = {k: np.asarray(v) for k, v in inputs.items()}
    if np.any(inputs["attention_mask"]):
        return _reference_fallback(**inputs)
    out, _ = _run(inputs)
    return out


# revision 45
# speedup vs baseline: 1.0688x; 1.0688x over previous
"""BartLatentAttention Trainium2 kernel.

Full-input contract: kernel(**inputs) takes the unsharded tensors from
setup_inputs() and returns the full [B, T, D] float32 output.

Sharding: tensor-parallel over heads. 16 heads / 8 cores = 2 heads per
core. Each core computes q/k/v projections for its 2 heads (column-sliced
weights), attention over the latent-prefixed sequence, and a partial
output projection (row-sliced Wo). The host sums the 8 partial outputs
and adds bo.

Device-side layout notes:
  - hidden is fed pre-transposed as hT [D, B*T] bf16 so projections can
    stream it as the moving matmul operand (contraction over D on the
    partition axis).
  - scores are computed transposed (scoresT [s, t]) so that softmax's
    exp can run on ScalarE straight out of PSUM, and the AV matmul can
    consume expT as the moving operand with V [s, d] stationary. The two
    heads' score matmuls land on PE row-tiles (0,0)/(64,0) and execute
    concurrently.
  - V carries an extra ones-column (M=65): PSUM row 64 of the AV
    accumulation is the softmax denominator Z for free.
  - S = L + T = 2056 is laid out padded to 2176 = 17*128: chunk 0 holds
    the 8 latent positions + 120 dead rows (killed with an exp bias of
    -30), chunks 1..16 hold the 2048 token positions.
  - epilogue is all-bf16 (zb broadcast matmul, ot, wo) and the final
    out-projection tiles are DMA'd directly from PSUM to DRAM.
  - ~28 junk 128-col matmuls at t=0 trip the PE HAM activity monitor so
    the real work starts at 2.4 GHz instead of 1.2 GHz.
"""

import sys

if "/opt/trn_rl_repo" not in sys.path:
    sys.path.insert(0, "/opt/trn_rl_repo")

import numpy as np
import ml_dtypes

BF16 = ml_dtypes.bfloat16

B, T, D = 2, 2048, 1024
H = 16
HD = D // H  # 64
L = 8
S = L + T  # 2056
SCALE = HD ** -0.5
NCORES = 8
HPC = H // NCORES  # heads per core = 2
DC = HPC * HD  # per-core feature width = 128

BT = B * T  # 4096
NKC = D // 128  # k chunks = 8
NTC = BT // 512  # token chunks of 512 = 8
SCHUNKS = 17  # padded S = 2176 = 17 * 128
TB = 512  # attention t-block
NTB = T // TB  # 4 per batch
PAD_BIAS = -30.0

_cache: dict = {}


def _build_nc():
    import concourse.bass as bass
    import concourse.mybir as mybir
    import concourse.tile as tile
    from concourse import bacc

    fp32 = mybir.dt.float32
    bf16 = mybir.dt.bfloat16

    nc = bacc.Bacc(
        "TRN2",
        target_bir_lowering=False,
        debug=False,
        enable_asserts=False,
        num_devices=NCORES,
    )

    # DRAM I/O
    hT = nc.dram_tensor("hT", [D, BT], bf16, kind="ExternalInput").ap()
    wq = nc.dram_tensor("wq", [D, DC], bf16, kind="ExternalInput").ap()
    wk = nc.dram_tensor("wk", [D, DC], bf16, kind="ExternalInput").ap()
    wv = nc.dram_tensor("wv", [D, DC], bf16, kind="ExternalInput").ap()
    bq = nc.dram_tensor("bq", [DC, 1], fp32, kind="ExternalInput").ap()
    bk = nc.dram_tensor("bk", [DC, 1], fp32, kind="ExternalInput").ap()
    bv1 = nc.dram_tensor("bv1", [DC, 1], fp32, kind="ExternalInput").ap()
    wo = nc.dram_tensor("wo", [DC, D], bf16, kind="ExternalInput").ap()
    lkT = nc.dram_tensor("lkT", [B, DC, L], bf16, kind="ExternalInput").ap()
    lv = nc.dram_tensor("lv", [B, HPC, L, HD], bf16, kind="ExternalInput").ap()
    ebias0 = nc.dram_tensor("ebias0", [128, 1], fp32, kind="ExternalInput").ap()
    e2 = nc.dram_tensor("e2", [2, 128], bf16, kind="ExternalInput").ap()
    fp16 = mybir.dt.float16
    out = nc.dram_tensor("out", [BT, D], fp16, kind="ExternalOutput").ap()

    EXP = mybir.ActivationFunctionType.Exp

    with tile.TileContext(nc) as tc:
        with (
            tc.tile_pool(name="consts", bufs=1) as consts,
            tc.tile_pool(name="persist", bufs=1) as persist,
            tc.tile_pool(name="htiles", bufs=2) as htiles,
            tc.tile_pool(name="exps", bufs=6) as exps,
            tc.tile_pool(name="episb", bufs=2) as episb,
            tc.tile_pool(name="scps", bufs=2, space="PSUM") as scps,
            tc.tile_pool(name="avp", bufs=2, space="PSUM") as avp,
            tc.tile_pool(name="mps", bufs=2, space="PSUM") as mps,
        ):
            # ---- PE warm-up: identity + junk matmuls, no DMA deps ----
            ident = consts.tile([128, 128], bf16)
            from concourse.masks import make_identity
            make_identity(nc, ident)
            # ~40 junk matmuls ≈ 4.3us of cold-clock busy: trips the HAM
            # activity monitor so the real phase-1 work runs at 2.4 GHz
            dum = mps.tile([128, 512], fp32, tag="mm", name="dum")
            for i in range(40):
                nc.tensor.matmul(dum[:, 0:128], ident, ident,
                                 start=True, stop=True)

            # ---- constants (DMA order matters: qkv weights + first h
            # chunk first, wo last) ----
            wq_sb = consts.tile([128, NKC * DC], bf16)  # [128, 8*128]
            wk_sb = consts.tile([128, NKC * DC], bf16)
            wv_sb = consts.tile([128, NKC * DC], bf16)
            wq_v = wq_sb.rearrange("p (k c) -> p k c", k=NKC)
            wk_v = wk_sb.rearrange("p (k c) -> p k c", k=NKC)
            wv_v = wv_sb.rearrange("p (k c) -> p k c", k=NKC)
            bq_sb = consts.tile([DC, 1], fp32)
            bk_sb = consts.tile([DC, 1], fp32)
            bv1_sb = consts.tile([DC, 1], fp32)
            wo_sb = consts.tile([DC, D], bf16)
            eb0_sb = consts.tile([128, 1], fp32)
            e2_sb = consts.tile([2, 128], bf16)
            # weights ride the otherwise-idle scalar DMA ring; small
            # constants go on gpsimd; the big hT loads are split in half
            # across the sync+gpsimd rings
            nc.scalar.dma_start(out=wq_v, in_=wq.rearrange("(k p) c -> p k c", p=128))
            nc.scalar.dma_start(out=eb0_sb, in_=ebias0)
            nc.scalar.dma_start(out=wk_v, in_=wk.rearrange("(k p) c -> p k c", p=128))
            nc.scalar.dma_start(out=wv_v, in_=wv.rearrange("(k p) c -> p k c", p=128))
            nc.gpsimd.dma_start(out=bq_sb, in_=bq)
            nc.gpsimd.dma_start(out=bk_sb, in_=bk)
            nc.gpsimd.dma_start(out=bv1_sb, in_=bv1)
            nc.gpsimd.dma_start(out=e2_sb, in_=e2)

            # ---- persistent activations ----
            qT_sb = persist.tile([128, BT], bf16)  # [h0|h1 feats, global tok]
            kT_sb = persist.tile([128, B * SCHUNKS * 128], bf16)  # per b: 2176
            v_sb = persist.tile([128, B * HPC * SCHUNKS * 65], bf16)

            def k_off(b):
                return b * SCHUNKS * 128

            def v_off(b, h, c):
                return ((b * HPC + h) * SCHUNKS + c) * 65

            # latent / pad setup
            for b in range(B):
                nc.vector.memset(kT_sb[:, k_off(b) + L:k_off(b) + 128], 0.0)
                nc.gpsimd.dma_start(out=kT_sb[:, k_off(b):k_off(b) + L],
                                    in_=lkT[b])
                for h in range(HPC):
                    nc.vector.memset(
                        v_sb[:, v_off(b, h, 0):v_off(b, h, 0) + 65], 0.0)
            # ones column for the Z fold (col 64 of every [128, 65] chunk)
            v_view = v_sb.rearrange("p (n c) -> p n c", c=65)
            nc.vector.memset(v_view[:, :, 64:65], 1.0)
            for b in range(B):
                for h in range(HPC):
                    nc.gpsimd.dma_start(
                        out=v_sb[0:L, v_off(b, h, 0):v_off(b, h, 0) + HD],
                        in_=lv[b, h])

            # wo loads on the scalar ring behind the qkv weights (not
            # needed until the first epilogue)
            def load_wo():
                nc.scalar.dma_start(out=wo_sb, in_=wo)

            # ---- qkv projection closures for one 512-token chunk ----
            def qkv_closures(g):
                t0g = g * 512
                bb = t0g // T
                c0 = (t0g - bb * T) // 128 + 1
                hold = {}

                def ht_load():
                    ht = htiles.tile([128, NKC, 512], bf16, tag="ht",
                                     name=f"ht_{g}")
                    # k-chunks 0-3 are consumed first (qa/ka/va); load the
                    # two halves on different DMA rings so each g-chunk
                    # lands in half the time. The scalar ring helps out in
                    # phase 1 while it has no activations to run.
                    e0, e1 = {0: (nc.sync, nc.gpsimd),
                              1: (nc.scalar, nc.sync),
                              2: (nc.gpsimd, nc.scalar),
                              3: (nc.sync, nc.gpsimd),
                              4: (nc.gpsimd, nc.sync),
                              5: (nc.sync, nc.gpsimd),
                              6: (nc.gpsimd, nc.sync),
                              7: (nc.sync, nc.gpsimd)}[g]
                    src = hT[:, t0g:t0g + 512].rearrange(
                        "(k p) t -> p k t", p=128)
                    e0.dma_start(out=ht[:, 0:4, :], in_=src[:, 0:4, :])
                    e1.dma_start(out=ht[:, 4:8, :], in_=src[:, 4:8, :])
                    hold["ht"] = ht

                def mk_proj(key, w_v):
                    def pa():
                        ps = mps.tile([128, 512], fp32, tag="mm",
                                      name=f"{key}ps_{g}")
                        hold[key] = ps
                        for k in range(4):
                            nc.tensor.matmul(
                                ps, w_v[:, k, :], hold["ht"][:, k, :],
                                start=(k == 0), stop=False)

                    def pb():
                        ps = hold[key]
                        for k in range(4, NKC):
                            nc.tensor.matmul(
                                ps, w_v[:, k, :], hold["ht"][:, k, :],
                                start=False, stop=(k == NKC - 1))
                    return pa, pb

                qa, qb = mk_proj("q", wq_v)
                ka, kb = mk_proj("k", wk_v)
                va, vb = mk_proj("v", wv_v)

                def q_fin():
                    nc.vector.tensor_scalar_add(
                        qT_sb[:, t0g:t0g + 512], hold["q"], bq_sb)

                def k_fin():
                    koff = k_off(bb) + 128 + (t0g - bb * T)
                    nc.vector.tensor_scalar_add(
                        kT_sb[:, koff:koff + 512], hold["k"], bk_sb)

                def v_fin():
                    vt = episb.tile([128, 512], bf16, tag="vt",
                                    name=f"vt_{g}")
                    nc.vector.tensor_scalar_add(vt, hold["v"], bv1_sb)
                    hold["vt"] = vt

                def t_a():
                    tp = mps.tile([128, 512], bf16, tag="mm",
                                  name=f"tp_{g}")
                    hold["tp"] = tp
                    for j in range(2):
                        nc.tensor.transpose(
                            tp[:, j * 128:(j + 1) * 128],
                            hold["vt"][:, j * 128:(j + 1) * 128], ident)

                def t_b():
                    tp = hold["tp"]
                    for j in range(2, 4):
                        nc.tensor.transpose(
                            tp[:, j * 128:(j + 1) * 128],
                            hold["vt"][:, j * 128:(j + 1) * 128], ident)
                    # v_sb[:, (c0+m, h, d)] = tp[:, (m, h, d)]
                    dst = bass.AP(
                        tensor=v_sb.tensor,
                        offset=v_sb.offset + v_off(bb, 0, c0),
                        ap=[v_sb.ap[0], [65, 4], [SCHUNKS * 65, HPC],
                            [1, HD]])
                    srcv = tp.rearrange("p (m e) -> p m e", m=4)
                    src = bass.AP(
                        tensor=srcv.tensor, offset=srcv.offset,
                        ap=[srcv.ap[0], [128, 4], [64, HPC], [1, HD]])
                    nc.vector.tensor_copy(dst, src)

                return [ht_load, qa, qb, q_fin, ka, kb, k_fin,
                        va, vb, v_fin, t_a, t_b]

            # ---- attention helpers ----
            def emit_epi_drain(st):
                av0, av1, tw = st["av0"], st["av1"], st["tw"]
                oz = episb.tile([128, 512], bf16, tag="oz",
                                name=f"oz_{st['q0']}")
                zr2 = episb.tile([2, 512], bf16, tag="zr2",
                                 name=f"zr2_{st['q0']}")
                zh0 = episb.tile([1, 512], fp32, tag="zh0",
                                 name=f"zh0_{st['q0']}")
                zh1 = episb.tile([1, 512], fp32, tag="zh1",
                                 name=f"zh1_{st['q0']}")
                rz1 = episb.tile([1, 512], bf16, tag="rz1",
                                 name=f"rz1_{st['q0']}")
                nc.vector.tensor_copy(oz[0:64, :tw], av0[0:64, :tw])
                nc.vector.tensor_copy(oz[64:128, :tw], av1[0:64, :tw])
                nc.vector.tensor_copy(zh0[:, :tw], av0[64:65, :tw])
                nc.vector.tensor_copy(zh1[:, :tw], av1[64:65, :tw])
                nc.vector.reciprocal_approx_fast(out=zh0[:, :tw],
                                                 in_=zh0[:, :tw])
                nc.vector.reciprocal_approx_fast(out=zh1[:, :tw],
                                                 in_=zh1[:, :tw])
                # DVE writes must start at a 32-aligned partition; row 1 of
                # zr2 goes through a DMA instead
                nc.vector.tensor_copy(zr2[0:1, :tw], zh0[:, :tw])
                nc.vector.tensor_copy(rz1[:, :tw], zh1[:, :tw])
                nc.gpsimd.dma_start(out=zr2[1:2, :tw], in_=rz1[:, :tw])
                st["oz"], st["zr2"] = oz, zr2

            def emit_epi_zb(st):
                tw = st["tw"]
                zb = mps.tile([128, 512], fp32, tag="mm",
                              name=f"zb_{st['q0']}")
                nc.tensor.matmul(zb[:, :tw], e2_sb, st["zr2"][:, :tw],
                                 start=True, stop=True)
                ot = episb.tile([128, 512], bf16, tag="ot",
                                name=f"ot_{st['q0']}")
                nc.vector.tensor_mul(ot[:, :tw], st["oz"][:, :tw],
                                     zb[:, :tw])
                st["ot"] = ot

            def mk_epi_out(st, j):
                def go():
                    if j >= st["tw"] // 128:
                        return
                    ot, q0 = st["ot"], st["q0"]
                    r0 = q0 + j * 128
                    for f in range(2):
                        op = mps.tile([128, 512], fp32, tag="mm",
                                      name=f"op_{q0}_{j}_{f}")
                        nc.tensor.matmul(
                            op, ot[:, j * 128:(j + 1) * 128],
                            wo_sb[:, f * 512:(f + 1) * 512],
                            start=True, stop=True)
                        osb = episb.tile([128, 512], fp16, tag="osb",
                                         name=f"osb_{q0}_{j}_{f}")
                        nc.vector.tensor_copy(osb, op)
                        nc.sync.dma_start(
                            out=out[r0:r0 + 128, f * 512:(f + 1) * 512],
                            in_=osb)
                return go

            def emit_av(st, c):
                b, tw = st["b"], st["tw"]
                stt, sp = c == 0, c == SCHUNKS - 1
                ex = st["ex"].pop(c)
                for h, av in ((0, st["av0"]), (1, st["av1"])):
                    vo = v_off(b, h, c)
                    eh = ex[:, h * tw:(h + 1) * tw]
                    nc.tensor.matmul(
                        av[:, :tw], v_sb[:, vo:vo + 65], eh,
                        start=stt, stop=sp)

            def make_st(b, q0, tw=TB):
                return {
                    "b": b, "q0": q0, "tw": tw,
                    "av0": avp.tile([65, 512], fp32, tag="avp",
                                    name=f"av0_{q0}"),
                    "av1": avp.tile([65, 512], fp32, tag="avp",
                                    name=f"av1_{q0}"),
                    "ex": {},
                }

            def chunk_body(st, c):
                b, q0, tw = st["b"], st["q0"], st["tw"]
                sc = scps.tile([128, 1024], fp32, tag="sc",
                               name=f"sc_{b}_{q0}_{c}")
                kc = k_off(b) + c * 128
                nc.tensor.matmul(
                    sc[:, 0:tw],
                    kT_sb[0:64, kc:kc + 128],
                    qT_sb[0:64, q0:q0 + tw],
                    start=True, stop=True)
                nc.tensor.matmul(
                    sc[:, tw:2 * tw],
                    kT_sb[64:128, kc:kc + 128],
                    qT_sb[64:128, q0:q0 + tw],
                    start=True, stop=True)
                ex = exps.tile([128, 1024], bf16, tag="ex",
                               name=f"ex_{b}_{q0}_{c}")
                nc.scalar.activation(
                    ex[:, 0:2 * tw], sc[:, 0:2 * tw], EXP,
                    bias=(eb0_sb if c == 0 else 0.0), scale=1.0)
                st["ex"][c] = ex
                if c >= 1:
                    emit_av(st, c - 1)

            def queue_epilogue(st, side):
                emit_av(st, SCHUNKS - 1)
                emit_epi_drain(st)
                # queued to run inside the next tb; two no-op slots let
                # the drain chain finish before zb consumes zr2

                def mk_zb(s):
                    def go():
                        emit_epi_zb(s)
                    return go
                noop = lambda: None
                for cl in reversed([noop, noop, mk_zb(st),
                                    mk_epi_out(st, 0),
                                    mk_epi_out(st, 1),
                                    mk_epi_out(st, 2),
                                    mk_epi_out(st, 3)]):
                    side.appendleft(cl)

            # ---- phase 1: qkv for batch 0, interleaved with the first
            # t-block's attention chunks (each chunk's K/V comes from the
            # previous g-chunk, so tb0 trails the projections by one) ----
            from collections import deque
            side = deque()
            for cl in qkv_closures(0):
                cl()
            load_wo()
            st0 = make_st(0, 0)
            chunk_body(st0, 0)
            for g in (1, 2, 3):
                cls = qkv_closures(g)
                base = 4 * (g - 1)
                for i in range(4):
                    chunk_body(st0, base + 1 + i)
                    for cl in cls[3 * i:3 * i + 3]:
                        cl()
            for c in range(13, SCHUNKS):
                chunk_body(st0, c)
            queue_epilogue(st0, side)
            for g in range(NTC // 2, NTC):
                side.extend(qkv_closures(g))

            # ---- phase 2: remaining t-blocks with interleaved side work
            # (tb0's epilogue, qkv for batch 1, later epilogues). The very
            # last t-block is split in half so most of its epilogue
            # overlaps the second half's chunk pipeline. ----
            schedule = ([(0, q, TB) for q in range(TB, T, TB)] +
                        [(1, T + q, TB) for q in range(0, T, TB)])
            for b, q0, tw in schedule:
                st = make_st(b, q0, tw)
                for c in range(SCHUNKS):
                    chunk_body(st, c)
                    if c >= 1 and side:
                        side.popleft()()
                        # drain two at a time while the backlog is
                        # long so batch 1's K/V is ready in time
                        if len(side) > 26 and side:
                            side.popleft()()
                queue_epilogue(st, side)
            # flush remaining side work (the last epilogues)
            while side:
                side.popleft()()

    nc.compile()
    return nc


def _get_nc():
    if "nc" not in _cache:
        _cache["nc"] = _build_nc()
    return _cache["nc"]


def _prep_inputs(hidden_states, decoder_latent, Wq, bq, Wk, bk, Wv, bv, Wo):
    """Build the 8 per-core input maps (host-side sharding/layout)."""
    hT = np.ascontiguousarray(
        hidden_states.reshape(BT, D).T).astype(BF16)
    lk = decoder_latent[..., :HD]  # [B, H, L, HD]
    lvf = decoder_latent[..., HD:]
    eb0 = np.full((128, 1), PAD_BIAS, np.float32)
    eb0[:L] = 0.0
    e2 = np.zeros((2, 128), np.float32)
    e2[0, 0:64] = 1.0
    e2[1, 64:128] = 1.0
    in_maps = []
    for c in range(NCORES):
        cols = slice(c * DC, (c + 1) * DC)
        h0, h1 = HPC * c, HPC * c + 1
        lkT_c = np.stack([
            np.concatenate([lk[b, h0].T, lk[b, h1].T], axis=0)
            for b in range(B)])  # [B, 128, L]
        in_maps.append({
            "hT": hT,
            "wq": (Wq[:, cols] * SCALE).astype(BF16),
            "wk": Wk[:, cols].astype(BF16),
            "wv": Wv[:, cols].astype(BF16),
            "bq": (bq[cols] * SCALE).astype(np.float32).reshape(DC, 1),
            "bk": bk[cols].astype(np.float32).reshape(DC, 1),
            "bv1": bv[cols].astype(np.float32).reshape(DC, 1),
            "wo": Wo[cols, :].astype(BF16),
            "lkT": lkT_c.astype(BF16),
            "lv": lvf[:, h0:h1 + 1].astype(BF16),
            "ebias0": eb0,
            "e2": e2.astype(BF16),
        })
    return in_maps


def _run(inputs, trace=False):
    from concourse.bass_utils import run_bass_kernel_spmd

    nc = _get_nc()
    in_maps = _prep_inputs(
        inputs["hidden_states"], inputs["decoder_latent"],
        inputs["Wq"], inputs["bq"], inputs["Wk"], inputs["bk"],
        inputs["Wv"], inputs["bv"], inputs["Wo"])
    res = run_bass_kernel_spmd(nc, in_maps, core_ids=list(range(NCORES)),
                               trace=trace)
    acc = np.zeros((BT, D), np.float64)
    for r in res.results:
        acc += r["out"].astype(np.float64)
    out = (acc + inputs["bo"].astype(np.float64)).astype(np.float32)
    return out.reshape(B, T, D), res


def _reference_fallback(hidden_states, decoder_latent, attention_mask,
                        Wq, bq, Wk, bk, Wv, bv, Wo, bo):
    """Exact numpy path, used only when attention_mask is non-zero (the
    problem spec fills it with zeros; the device kernel specializes on
    that)."""
    x = hidden_states.astype(np.float64)
    q = (x @ Wq + bq) * SCALE
    k = x @ Wk + bk
    v = x @ Wv + bv

    def heads(a):
        return a.reshape(B, T, H, HD).transpose(0, 2, 1, 3)

    q, k, v = heads(q), heads(k), heads(v)
    lk = decoder_latent[..., :HD].astype(np.float64)
    lv = decoder_latent[..., HD:].astype(np.float64)
    k = np.concatenate([lk, k], axis=2)
    v = np.concatenate([lv, v], axis=2)
    s = np.einsum("bhtd,bhsd->bhts", q, k) + attention_mask.astype(np.float64)
    s -= s.max(axis=-1, keepdims=True)
    p = np.exp(s)
    p /= p.sum(axis=-1, keepdims=True)
    o = np.einsum("bhts,bhsd->bhtd", p, v)
    o = o.transpose(0, 2, 1, 3).reshape(B, T, D)
    return (o @ Wo + bo).astype(np.float32)


def kernel(**inputs):
    inputs = {k: np.asarray(v) for k, v in inputs.items()}
    if np.any(inputs["attention_mask"]):
        return _reference_fallback(**inputs)
    out, _ = _run(inputs)
    return out
